# revision 6
# baseline (speedup 1.0000x reference)
"""Trainium2 Bass kernel for the CNF reversible backward solve.

Math restructuring (exact, validated in fp64 against the jax reference):

The per-step recursion
    f1 = W2 tanh(W1 y + b(t1)) + b2
    z' = z - h f1
    f0 = W2 tanh(W1 z' + b(t0)) + b2
    y' = inv_l y + (1-inv_l) z' - inv_l h f0
is tracked purely in H-space (H=256) via Zs = W1 z, Ys = W1 y:
    a_even = tanh(Y + beta_even)        [scalar engine, per-partition bias]
    Z     += Mz @ a_even                [PSUM-resident, Mz = -h W1 W2]
    a_odd  = tanh(Z + beta_odd)
    Y'     = inv_l Y + (1-inv_l) Z + inv_l (Mz @ a_odd)  [one fused DVE op]
The D-space outputs never enter the loop:
    y_final = c_y y1 + sum_e gamma_e (W2 @ a_e) + c_b b2
    I_final = h (N * sum(c) - sum_s c . a_even_s^2),  c = diag(W1 W2)
with exact coefficients gamma_e / c_y / c_b derived on the host, and the
weighted sums accumulated on-device in PSUM via pre-scaled gamma_e*W2^T
weight tables.

Sharding: data-parallel, B=256 -> 32 samples on each of 8 cores; all
parameters/tables replicated. Output gathered and assembled on host.
"""

import numpy as np
import ml_dtypes
from contextlib import ExitStack

import concourse.bass as bass
import concourse.tile as tile
from concourse import bacc, mybir
from concourse.bass_utils import run_bass_kernel_spmd

# Problem constants (hardcoded per contract)
NCORES = 8
B, D, H = 256, 64, 256
NSTEP = 64
HSTEP = 1.0 / NSTEP
LCOUP = 0.999
INVL = 1.0 / LCOUP
BS = B // NCORES  # 32 samples per core
NBLK = H // 128  # 2 h-blocks
FREE = NBLK * BS  # 64: free size of H-space tiles, layout (blk, sample)
NEVAL = 2 * NSTEP  # 128

F32 = mybir.dt.float32
BF16 = mybir.dt.bfloat16
BF16NP = ml_dtypes.bfloat16


def _coefficients():
    """Exact fp64 scalar recursions for the output-extraction weights."""
    gamma = np.zeros(NEVAL)
    la = np.zeros(NEVAL)
    alpha_y = alpha_z = 1.0
    nu_y = nu_z = 0.0
    for s in range(NSTEP):
        la[2 * s] += -HSTEP
        nu_z += -HSTEP
        gamma *= INVL
        alpha_y *= INVL
        nu_y *= INVL
        gamma += (1.0 - INVL) * la
        alpha_y += (1.0 - INVL) * alpha_z
        nu_y += (1.0 - INVL) * nu_z
        gamma[2 * s + 1] += -INVL * HSTEP
        nu_y += -INVL * HSTEP
    return gamma, alpha_y, nu_y


def _host_tables(W1, b1, u1, W2, b2):
    """All precomputed tensors, fp64 internally."""
    W1 = W1.astype(np.float64)
    W2 = W2.astype(np.float64)
    b1 = b1.astype(np.float64)
    u1 = u1.astype(np.float64)
    b2 = b2.astype(np.float64)

    gamma, c_y, c_b = _coefficients()
    Mz = -HSTEP * (W1 @ W2)  # [H, H]
    W1b2 = W1 @ b2  # [H]
    cvec = np.sum(W1 * W2.T, axis=1)  # diag(W1@W2)

    kappa = INVL ** (-np.arange(NSTEP + 1, dtype=np.float64))
    d = np.zeros((NSTEP + 1, H))
    for s in range(NSTEP):
        d[s + 1] = d[s] - kappa[s] * HSTEP * W1b2

    # mzt_pack[p, (k*NBLK+j)*128 + m] = Mz[128*j+m, 128*k+p]
    MzT = Mz.T  # [K(h), M(h')]
    mzt_pack = np.zeros((128, NBLK * NBLK * 128))
    for k in range(NBLK):
        for j in range(NBLK):
            mzt_pack[:, (k * NBLK + j) * 128 : (k * NBLK + j + 1) * 128] = MzT[
                128 * k : 128 * k + 128, 128 * j : 128 * j + 128
            ]

    # w2g_pack[p, (e*NBLK+k)*D + m] = gamma[e] * W2[m, 128*k+p]
    W2T = W2.T  # [H, D]
    w2g_pack = np.zeros((128, NEVAL * NBLK * D))
    for e in range(NEVAL):
        for k in range(NBLK):
            w2g_pack[:, (e * NBLK + k) * D : (e * NBLK + k + 1) * D] = (
                gamma[e] * W2T[128 * k : 128 * k + 128, :]
            )

    # bias tables [128, NSTEP*NBLK], col s*NBLK+blk
    be_pack = np.zeros((128, NSTEP * NBLK))
    bo_pack = np.zeros((128, NSTEP * NBLK))
    for s in range(NSTEP):
        t1 = 1.0 - s * HSTEP
        t0 = 1.0 - (s + 1) * HSTEP
        bias_even = b1 + t1 * u1 - (1.0 / kappa[s]) * d[s]
        bias_odd = b1 + t0 * u1 - (s + 1) * HSTEP * W1b2
        for blk in range(NBLK):
            be_pack[:, s * NBLK + blk] = bias_even[128 * blk : 128 * blk + 128]
            bo_pack[:, s * NBLK + blk] = bias_odd[128 * blk : 128 * blk + 128]

    ct_pack = np.zeros((128, NBLK))
    for k in range(NBLK):
        ct_pack[:, k] = cvec[128 * k : 128 * k + 128]

    w1t_pack = W1.T  # [D, H]

    scalars = dict(
        kappa=kappa,
        c_y=c_y,
        c_b=c_b,
        sum_c=float(np.sum(cvec)),
    )
    return dict(
        mzt=mzt_pack.astype(BF16NP),
        w2g=w2g_pack.astype(BF16NP),
        be=be_pack.astype(np.float32),
        bo=bo_pack.astype(np.float32),
        ct=ct_pack.astype(BF16NP),
        w1t=w1t_pack.astype(np.float32),
        scalars=scalars,
    )


def _build_kernel():
    """Build the Bass module (same program for every core)."""
    nc = bacc.Bacc("TRN2", target_bir_lowering=False, debug=False)

    y1t_d = nc.dram_tensor("y1t", [D, BS], F32, kind="ExternalInput").ap()
    w1t_d = nc.dram_tensor("w1t", [D, H], F32, kind="ExternalInput").ap()
    mzt_d = nc.dram_tensor("mzt", [128, NBLK * NBLK * 128], BF16, kind="ExternalInput").ap()
    w2g_d = nc.dram_tensor("w2g", [128, NEVAL * NBLK * D], BF16, kind="ExternalInput").ap()
    be_d = nc.dram_tensor("be", [128, NSTEP * NBLK], F32, kind="ExternalInput").ap()
    bo_d = nc.dram_tensor("bo", [128, NSTEP * NBLK], F32, kind="ExternalInput").ap()
    ct_d = nc.dram_tensor("ct", [128, NBLK], BF16, kind="ExternalInput").ap()

    py_out_d = nc.dram_tensor("py_out", [D, BS], F32, kind="ExternalOutput").ap()
    ptr_out_d = nc.dram_tensor("ptr_out", [1, BS], F32, kind="ExternalOutput").ap()

    kappa = INVL ** (-np.arange(NSTEP + 1, dtype=np.float64))

    with tile.TileContext(nc) as tc, ExitStack() as ctx:
        consts = ctx.enter_context(tc.tile_pool(name="consts", bufs=1))
        psum = ctx.enter_context(tc.tile_pool(name="psum", bufs=1, space="PSUM"))
        gpool = ctx.enter_context(tc.tile_pool(name="gps", bufs=2, space="PSUM"))
        apool = ctx.enter_context(tc.tile_pool(name="acts", bufs=3))
        wpool = ctx.enter_context(tc.tile_pool(name="wstate", bufs=3))
        ppool = ctx.enter_context(tc.tile_pool(name="ptmp", bufs=2))
        qpool = ctx.enter_context(tc.tile_pool(name="qsq", bufs=2))
        opool = ctx.enter_context(tc.tile_pool(name="outs", bufs=1))

        # --- prime the tanh activation table early (dep-free) ---
        warm = consts.tile([1, 8], F32, tag="warm")
        nc.vector.memset(warm[:], 0.0)
        nc.scalar.activation(warm[:], warm[:], mybir.ActivationFunctionType.Tanh)

        # --- load constants ---
        y1t = consts.tile([D, BS], F32, tag="y1t")
        nc.sync.dma_start(y1t[:], y1t_d)
        w1t = consts.tile([D, H], F32, tag="w1t")
        nc.sync.dma_start(w1t[:], w1t_d)
        mzt = consts.tile([128, NBLK * NBLK * 128], BF16, tag="mzt")
        nc.sync.dma_start(mzt[:], mzt_d)
        be = consts.tile([128, NSTEP * NBLK], F32, tag="be")
        nc.sync.dma_start(be[:], be_d)
        bo = consts.tile([128, NSTEP * NBLK], F32, tag="bo")
        nc.sync.dma_start(bo[:], bo_d)
        ct = consts.tile([128, NBLK], BF16, tag="ct")
        nc.sync.dma_start(ct[:], ct_d)
        w2g = consts.tile([128, NEVAL * NBLK * D], BF16, tag="w2g")
        W2G_CHUNKS = 8
        ccols = NEVAL * NBLK * D // W2G_CHUNKS
        for c in range(W2G_CHUNKS):
            nc.sync.dma_start(
                w2g[:, c * ccols : (c + 1) * ccols], w2g_d[:, c * ccols : (c + 1) * ccols]
            )

        def mzt_blk(k, j):
            base = (k * NBLK + j) * 128
            return mzt[:, base : base + 128]

        def w2g_blk(e, k):
            base = (e * NBLK + k) * D
            return w2g[:, base : base + D]

        # --- persistent PSUM accumulators ---
        z_ps = psum.tile([128, FREE], F32, tag="z")
        py_ps = psum.tile([D, BS], F32, tag="py")
        ptr_ps = psum.tile([1, BS], F32, tag="ptr")

        # --- init: Z_0 = W1 @ y1 ---
        for j in range(NBLK):
            nc.tensor.matmul(
                z_ps[:, j * BS : (j + 1) * BS],
                w1t[:, 128 * j : 128 * j + 128],
                y1t[:],
                start=(j == 0),
                stop=False,
            )

        # W_0 = copy of Z_0
        w_st = wpool.tile([128, FREE], F32, tag="w")
        nc.vector.tensor_copy(w_st[:], z_ps[:])

        for s in range(NSTEP):
            e_even = 2 * s
            e_odd = 2 * s + 1
            last = s == NSTEP - 1

            # --- even eval: a_even = tanh(scale * W + bias_even) ---
            a_even = apool.tile([128, FREE], BF16, tag="a_even")
            scale_s = float(INVL**s)  # 1/kappa[s]
            for blk in range(NBLK):
                nc.scalar.activation(
                    a_even[:, blk * BS : (blk + 1) * BS],
                    w_st[:, blk * BS : (blk + 1) * BS],
                    mybir.ActivationFunctionType.Tanh,
                    bias=be[:, s * NBLK + blk : s * NBLK + blk + 1],
                    scale=scale_s,
                )

            # --- Z += Mz @ a_even ---
            for k in range(NBLK):
                for j in range(NBLK):
                    nc.tensor.matmul(
                        z_ps[:, j * BS : (j + 1) * BS],
                        mzt_blk(k, j),
                        a_even[:, k * BS : (k + 1) * BS],
                        start=False,
                        stop=(last and k == NBLK - 1 and j == NBLK - 1),
                    )

            # --- output accumulation for even eval ---
            for k in range(NBLK):
                nc.tensor.matmul(
                    py_ps[:],
                    w2g_blk(e_even, k),
                    a_even[:, k * BS : (k + 1) * BS],
                    start=(e_even == 0 and k == 0),
                    stop=False,
                )

            # --- trace: q = a_even^2 ; P_tr += c^T q ---
            q = qpool.tile([128, FREE], BF16, tag="q")
            nc.vector.tensor_tensor(q[:], a_even[:], a_even[:], mybir.AluOpType.mult)
            for k in range(NBLK):
                nc.tensor.matmul(
                    ptr_ps[:],
                    ct[:, k : k + 1],
                    q[:, k * BS : (k + 1) * BS],
                    start=(s == 0 and k == 0),
                    stop=(last and k == NBLK - 1),
                )

            # --- STT#1: p = c1_s * Z + W_s (DVE, overlaps odd ACT) ---
            if not last:
                p_t = ppool.tile([128, FREE], F32, tag="p")
                c1_s = float(kappa[s + 1] * (1.0 - INVL))
                nc.vector.scalar_tensor_tensor(
                    p_t[:], z_ps[:], c1_s, w_st[:],
                    mybir.AluOpType.mult, mybir.AluOpType.add,
                )

            # --- odd eval: a_odd = tanh(Z + bias_odd) ---
            a_odd = apool.tile([128, FREE], BF16, tag="a_odd")
            for blk in range(NBLK):
                nc.scalar.activation(
                    a_odd[:, blk * BS : (blk + 1) * BS],
                    z_ps[:, blk * BS : (blk + 1) * BS],
                    mybir.ActivationFunctionType.Tanh,
                    bias=bo[:, s * NBLK + blk : s * NBLK + blk + 1],
                    scale=1.0,
                )

            # --- G = Mz @ a_odd (fresh PSUM group each step; dead at last step) ---
            if not last:
                g_ps = gpool.tile([128, FREE], F32, tag="g")
                first_mm = True
                for k in range(NBLK):
                    for j in range(NBLK):
                        nc.tensor.matmul(
                            g_ps[:, j * BS : (j + 1) * BS],
                            mzt_blk(k, j),
                            a_odd[:, k * BS : (k + 1) * BS],
                            start=first_mm,
                            stop=(k == NBLK - 1 and j == NBLK - 1),
                        )
                        first_mm = False

            # --- output accumulation for odd eval ---
            for k in range(NBLK):
                nc.tensor.matmul(
                    py_ps[:],
                    w2g_blk(e_odd, k),
                    a_odd[:, k * BS : (k + 1) * BS],
                    start=False,
                    stop=(last and k == NBLK - 1),
                )

            # --- STT#2: W_{s+1} = g2_s * G + p ---
            if not last:
                w_new = wpool.tile([128, FREE], F32, tag="w")
                g2_s = float(kappa[s])
                nc.vector.scalar_tensor_tensor(
                    w_new[:], g_ps[:], g2_s, p_t[:],
                    mybir.AluOpType.mult, mybir.AluOpType.add,
                )
                w_st = w_new

        # --- drain outputs ---
        py_sb = opool.tile([D, BS], F32, tag="py_sb")
        nc.vector.tensor_copy(py_sb[:], py_ps[:])
        ptr_sb = opool.tile([1, BS], F32, tag="ptr_sb")
        nc.vector.tensor_copy(ptr_sb[:], ptr_ps[:])
        nc.sync.dma_start(py_out_d, py_sb[:])
        nc.sync.dma_start(ptr_out_d, ptr_sb[:])

    nc.compile()
    return nc


_CACHE = {}


def _get_kernel():
    if "nc" not in _CACHE:
        _CACHE["nc"] = _build_kernel()
    return _CACHE["nc"]


def kernel(y1, W1, b1, u1, W2, b2, _trace=False, _trace_kwargs=None):
    y1 = np.asarray(y1)
    in_dtype = y1.dtype
    tabs = _host_tables(
        np.asarray(W1), np.asarray(b1), np.asarray(u1), np.asarray(W2), np.asarray(b2)
    )
    sc = tabs["scalars"]

    nc = _get_kernel()

    shared = {
        "w1t": tabs["w1t"],
        "mzt": tabs["mzt"],
        "w2g": tabs["w2g"],
        "be": tabs["be"],
        "bo": tabs["bo"],
        "ct": tabs["ct"],
    }
    in_maps = []
    for c in range(NCORES):
        shard = y1[c * BS : (c + 1) * BS].astype(np.float32)  # [BS, D]
        m = dict(shared)
        m["y1t"] = np.ascontiguousarray(shard.T)  # [D, BS]
        in_maps.append(m)

    kw = {}
    if _trace:
        kw["trace"] = True
        if _trace_kwargs:
            kw.update(_trace_kwargs)
    res = run_bass_kernel_spmd(nc, in_maps, core_ids=list(range(NCORES)), **kw)

    out = np.zeros((B, D + 1), dtype=np.float32)
    for c in range(NCORES):
        py = np.asarray(res.results[c]["py_out"], dtype=np.float64)  # [D, BS]
        ptr = np.asarray(res.results[c]["ptr_out"], dtype=np.float64)  # [1, BS]
        shard = y1[c * BS : (c + 1) * BS].astype(np.float64)
        y_fin = sc["c_y"] * shard + py.T + sc["c_b"] * np.asarray(b2, np.float64)[None, :]
        i_fin = HSTEP * (NSTEP * sc["sum_c"] - ptr[0])
        out[c * BS : (c + 1) * BS, :D] = y_fin.astype(np.float32)
        out[c * BS : (c + 1) * BS, D] = i_fin.astype(np.float32)

    if _trace:
        return out.astype(in_dtype, copy=False), res
    return out.astype(in_dtype, copy=False)
